# revision 9
# baseline (speedup 1.0000x reference)
"""Trainium2 Bass kernel for nn_Attention_36601711297049.

Self-attention (4 heads, dim_head 32) over N=4096 tokens, batch 2:
  qkv = w_qkv @ x ; sim = scale * q^T k ; attn = softmax(sim) ;
  out = attn @ v ; y = w_out @ out + b_out

Sharding: 8 cores = 2 batches x 4 query-chunks (1024 queries each).
Each core computes k, v for the full batch plus q for its own chunk, runs
flash-style attention in S^T layout ([keys, queries], so the AV contraction
needs no transposes), and applies the output projection locally. No
collectives. Softmax skips max-subtraction (logits are ~N(0,1), safely
inside fp32/exp range).

Key-order trick: the host permutes each core's x so that key-piece 0 IS the
core's query chunk — softmax/AV are key-order invariant, and the q gemm can
start as soon as the first DMA piece lands (no separate xq transfer).

Engine budget per core: ScalarE exp is the floor (~2.3us per 512-query x
128-key x 4-head iteration); the PE (S^T 4-tile block + AV col-pairs +
qkv gemms) hides underneath via a one-iteration AV lag.
"""
import sys

for p in ("/opt/trn_rl_repo", "/root/.axon_site/_ro/trn_rl_repo"):
    if p not in sys.path:
        sys.path.insert(0, p)

import numpy as np
from contextlib import ExitStack

import concourse.bass as bass
from concourse import bacc
import concourse.tile as tile
from concourse import mybir
from concourse.bass_utils import run_bass_kernel_spmd

F32 = mybir.dt.float32
BF16 = mybir.dt.bfloat16
AF = mybir.ActivationFunctionType

HEADS = 4
DH = 32
C = 256          # channels
N = 4096         # h*w tokens per batch
QC = 1024        # queries per core
NK = N // 128    # 128-key tiles
SCALE = float(DH) ** -0.5


def build_nc():
    nc = bacc.Bacc("TRN2", target_bir_lowering=False)
    x = nc.dram_tensor("x", [C, N], BF16, kind="ExternalInput")
    wq = nc.dram_tensor("wq", [128, 2, 128], BF16, kind="ExternalInput")  # [p, cc, (h,d)]
    wk = nc.dram_tensor("wk", [128, 2, 128], BF16, kind="ExternalInput")
    wv = nc.dram_tensor("wv", [128, 2, 128], BF16, kind="ExternalInput")
    woA = nc.dram_tensor("woA", [128, C], BF16, kind="ExternalInput")  # w_out^T h0/h1 + bias row
    woB = nc.dram_tensor("woB", [128, C], BF16, kind="ExternalInput")  # w_out^T h2/h3, zero-padded
    out = nc.dram_tensor("out", [C, QC], F32, kind="ExternalOutput")

    with tile.TileContext(nc) as tc, ExitStack() as ctx:
        big = ctx.enter_context(tc.tile_pool(name="big", bufs=1))
        small = ctx.enter_context(tc.tile_pool(name="small", bufs=2))
        ptp = ctx.enter_context(tc.tile_pool(name="ptp", bufs=18))
        stp = ctx.enter_context(tc.tile_pool(name="stp", bufs=3, space="PSUM"))
        avp = ctx.enter_context(tc.tile_pool(name="avp", bufs=2, space="PSUM"))

        # warm the exp table set early (one tiny ACT forces the table load)
        dummy = small.tile([1, 8], F32, tag="dummy")
        nc.vector.memset(dummy[:], 0.0)
        nc.scalar.activation(dummy[:], dummy[:], AF.Exp)

        # ---- weights first (tiny, on sync), then x pieces on 4 queues ----
        wq_bf = big.tile([128, 2, 128], BF16, tag="wq_bf")
        wk_bf = big.tile([128, 2, 128], BF16, tag="wk_bf")
        wv_bf = big.tile([128, 2, 128], BF16, tag="wv_bf")
        for (dram, sbuf) in ((wq, wq_bf), (wk, wk_bf), (wv, wv_bf)):
            nc.sync.dma_start(sbuf[:], dram[:])
        woA_bf = big.tile([128, 256], BF16, tag="woA_bf")
        woB_bf = big.tile([128, 256], BF16, tag="woB_bf")
        nc.sync.dma_start(woA_bf[:], woA[:])
        nc.sync.dma_start(woB_bf[:], woB[:])
        ones_bf = big.tile([128, DH], BF16, tag="ones_bf")
        nc.vector.memset(ones_bf[:], 1.0)

        x_bf = big.tile([128, 2, N], BF16, tag="x_bf")
        dma_engines = (nc.sync, nc.gpsimd)

        def dma_x_piece(piece, di):
            # split each [128, 2, 1024] piece into cc halves on 2 queues
            sl = slice(1024 * piece, 1024 * (piece + 1))
            for cc in range(2):
                dma_engines[(di + cc) % 2].dma_start(
                    x_bf[:, cc, sl], x[128 * cc:128 * (cc + 1), sl])

        # piece 0 = this core's query chunk: 4 quarter-DMAs; the scalar
        # queue helps only here (exp stream hasn't started yet)
        p0_engines = (nc.sync, nc.gpsimd, nc.scalar, nc.gpsimd)
        for cc in range(2):
            for hh in range(2):
                sl = slice(512 * hh, 512 * (hh + 1))
                p0_engines[2 * cc + hh].dma_start(
                    x_bf[:, cc, sl], x[128 * cc:128 * (cc + 1), sl])

        for piece in range(1, 4):
            dma_x_piece(piece, 2 * piece)

        # ---- q = wq^T x[:, :, 0:QC] : [128 (h,d), QC] bf16 ----
        q_bf = big.tile([128, QC], BF16, tag="q_bf")
        for nch in range(QC // 512):
            ps = stp.tile([128, 1024], F32, tag="st", name=f"q_ps{nch}")
            for cc in range(2):
                nc.tensor.matmul(ps[:, :512], wq_bf[:, cc, :],
                                 x_bf[:, cc, 512 * nch:512 * (nch + 1)],
                                 start=(cc == 0), stop=(cc == 1),
                                 skip_group_check=True)
            nc.vector.tensor_copy(q_bf[:, 512 * nch:512 * (nch + 1)], ps[:, :512])

        # ---- k = wk^T x and vT = x^T wv, emitted piecewise, interleaved ----
        k_bf = big.tile([128, N], BF16, tag="k_bf")
        vT_bf = big.tile([128, NK, 4, 34], BF16, tag="vT_bf")
        for h in range(HEADS):
            nc.vector.memset(vT_bf[:, :, h, 32:33], 1.0)

        def emit_k_gemm(nch):
            ps = stp.tile([128, 1024], F32, tag="st", name=f"k_ps{nch}")
            for cc in range(2):
                nc.tensor.matmul(ps[:, :512], wk_bf[:, cc, :],
                                 x_bf[:, cc, 512 * nch:512 * (nch + 1)],
                                 start=(cc == 0), stop=(cc == 1),
                                 skip_group_check=True)
            nc.vector.tensor_copy(k_bf[:, 512 * nch:512 * (nch + 1)], ps[:, :512])

        def emit_vT_gemm(kt):
            ps = stp.tile([128, 1024], F32, tag="st", name=f"v_ps{kt}")
            for cc in range(2):
                nc.tensor.matmul(ps[:, :128], x_bf[:, cc, 128 * kt:128 * (kt + 1)],
                                 wv_bf[:, cc, :],
                                 start=(cc == 0), stop=(cc == 1),
                                 skip_group_check=True)
            nc.vector.tensor_copy(
                vT_bf[:, kt, :, 0:32],
                ps[:, :128].rearrange("p (h d) -> p h d", d=32))

        # ---- attention main loop ----
        avbs = {}
        pts_store = {}

        def emit_st_exp(qc, kt):
            """S^T 4-tile block + exp for (qc, kt)."""
            if qc == 0:
                if kt == 0:
                    for nch in range(3):
                        emit_k_gemm(nch)
                    for kk in range(5):
                        emit_vT_gemm(kk)
                else:
                    if kt % 4 == 0 and kt // 4 + 2 < N // 512:
                        emit_k_gemm(kt // 4 + 2)
                    if kt + 4 < NK:
                        emit_vT_gemm(kt + 4)
            qsl = slice(512 * qc, 512 * (qc + 1))
            st0 = stp.tile([128, 1024], F32, tag="st", name=f"st0_{qc}_{kt}")
            st1 = stp.tile([128, 1024], F32, tag="st", name=f"st1_{qc}_{kt}")
            sts = (st0, st0, st1, st1)
            for h in range(HEADS):
                nc.tensor.matmul(
                    sts[h][:, 512 * (h % 2):512 * (h % 2 + 1)],
                    k_bf[32 * h:32 * (h + 1), 128 * kt:128 * (kt + 1)],
                    q_bf[32 * h:32 * (h + 1), qsl],
                    start=True, stop=True, skip_group_check=True,
                    tile_position=(32 * h, 0))
            pt0 = ptp.tile([128, 1024], BF16, tag="pt", name=f"pt0_{qc}_{kt}")
            pt1 = ptp.tile([128, 1024], BF16, tag="pt", name=f"pt1_{qc}_{kt}")
            nc.scalar.activation(pt0[:], st0[:], AF.Exp, scale=SCALE)
            nc.scalar.activation(pt1[:], st1[:], AF.Exp, scale=SCALE)
            pts_store[(qc, kt)] = (pt0, pt1)

        def emit_av(qc, kt):
            if kt == 0:
                avbs[qc] = [avp.tile([128, 512], F32, tag="acc", name=f"av{qc}_{b}")
                            for b in range(2)]
            pt0, pt1 = pts_store.pop((qc, kt))
            pts = (pt0, pt0, pt1, pt1)
            # AV with ones column: M=33, out rows 0:33 / 64:97 per bank
            for h in range(HEADS):
                psl = slice(512 * (h % 2), 512 * (h % 2 + 1))
                half = h % 2
                nc.tensor.matmul(
                    avbs[qc][h // 2][64 * half:64 * half + 33, :],
                    vT_bf[:, kt, h, 0:33],
                    pts[h][:, psl],
                    start=(kt == 0), stop=(kt == NK - 1),
                    skip_group_check=True, tile_position=(0, 64 * half))

        def emit_epilogue(qc):
            qsl = slice(512 * qc, 512 * (qc + 1))
            avb = avbs[qc]
            hids = []
            for b in range(2):
                rec_f = small.tile([128, 512], F32, tag="rec_f", name=f"rec{qc}_{b}")
                nc.vector.reciprocal(rec_f[0:97, :], avb[b][0:97, :])
                rec_bf = small.tile([128, 512], BF16, tag="rec_bf", name=f"recb{qc}_{b}")
                nc.vector.tensor_copy(rec_bf[32:33, :], rec_f[32:33, :])
                nc.vector.tensor_copy(rec_bf[96:97, :], rec_f[96:97, :])
                bc = stp.tile([128, 1024], F32, tag="st", name=f"bc{qc}_{b}")
                for half in range(2):
                    r = 64 * half + 32
                    nc.tensor.matmul(bc[64 * half:64 * half + 32, :512],
                                     ones_bf[r:r + 1, 0:32], rec_bf[r:r + 1, :],
                                     start=True, stop=True, skip_group_check=True,
                                     tile_position=(r - r % 32, 64 * half))
                bc_sb = small.tile([128, 512], F32, tag="bc_sb", name=f"bcs{qc}_{b}")
                hid = small.tile([128, 512], BF16, tag="hid", name=f"hid{qc}_{b}")
                nc.vector.memset(hid[32:64, :], 0.0)
                nc.vector.memset(hid[96:128, :], 0.0)
                if b == 0:
                    # ones row 32 of hids[0] picks up the bias row of woA
                    nc.vector.memset(hid[32:33, :], 1.0)
                for half in range(2):
                    rows = slice(64 * half, 64 * half + 32)
                    nc.vector.tensor_copy(bc_sb[rows, :], bc[rows, :512])
                    nc.vector.tensor_mul(hid[rows, :], avb[b][rows, :], bc_sb[rows, :])
                hids.append(hid)

            for oc in range(2):
                yt = stp.tile([128, 1024], F32, tag="st", name=f"y{qc}_{oc}")
                yps = yt[:, :512]
                nc.tensor.matmul(yps[:], woA_bf[:, 128 * oc:128 * (oc + 1)],
                                 hids[0][:], start=True, stop=False,
                                 skip_group_check=True)
                nc.tensor.matmul(yps[:], woB_bf[:, 128 * oc:128 * (oc + 1)],
                                 hids[1][:], start=False, stop=True,
                                 skip_group_check=True)
                ysb = small.tile([128, 512], F32, tag="ysb", name=f"ysb{qc}_{oc}")
                nc.vector.tensor_copy(ysb[:], yps[:])
                dma_engines[oc % 2].dma_start(out[128 * oc:128 * (oc + 1), qsl], ysb[:])

        # software pipeline: AV lags ST/exp by one step so the PE queue
        # never blocks on the scalar engine; qc1's first PIPE AVs are
        # deferred past qc0's epilogue (they reuse its PSUM banks), and the
        # exp stream keeps running through the epilogue regardless.
        PIPE = 6
        emit_st_exp(0, 0)
        for kt in range(1, NK):
            emit_st_exp(0, kt)
            emit_av(0, kt - 1)
        emit_st_exp(1, 0)
        emit_av(0, NK - 1)
        for kt in range(1, PIPE + 1):
            emit_st_exp(1, kt)
        emit_epilogue(0)
        for j in range(PIPE):
            emit_av(1, j)
        for kt in range(PIPE + 1, NK):
            emit_st_exp(1, kt)
            emit_av(1, kt - 1)
        emit_av(1, NK - 1)
        emit_epilogue(1)
    return nc


_NC_CACHE = None


def _get_nc():
    global _NC_CACHE
    if _NC_CACHE is None:
        nc = build_nc()
        nc.compile()
        _NC_CACHE = nc
    return _NC_CACHE


def _prep_weights(w_qkv, w_out, b_out):
    # w_qkv rows are interleaved: row (h*32+d)*3 + {0:q, 1:k, 2:v}
    w = np.asarray(w_qkv, np.float32).reshape(HEADS, DH, 3, C)
    import ml_dtypes

    def to_pcc(m):   # [C, 128] -> [p, cc, 128] bf16
        return np.ascontiguousarray(
            m.reshape(2, 128, 128).transpose(1, 0, 2)).astype(ml_dtypes.bfloat16)
    wq = to_pcc(w[:, :, 0, :].reshape(128, C).T)
    wk = to_pcc(w[:, :, 1, :].reshape(128, C).T)
    wv = to_pcc(w[:, :, 2, :].reshape(128, C).T)
    woT = np.asarray(w_out, np.float32).T                        # [128 c', C]
    woA = np.zeros((128, C), np.float32)
    woB = np.zeros((128, C), np.float32)
    woA[0:32] = woT[0:32]       # head 0
    woA[64:96] = woT[32:64]     # head 1
    woB[0:32] = woT[64:96]      # head 2
    woB[64:96] = woT[96:128]    # head 3
    woA[32] = np.asarray(b_out, np.float32)   # bias row (hid row 32 = ones)
    woA = woA.astype(ml_dtypes.bfloat16)
    woB = woB.astype(ml_dtypes.bfloat16)
    return wq, wk, wv, woA, woB


def kernel(x, w_qkv, w_out, b_out):
    import ml_dtypes
    x = np.asarray(x, np.float32)
    b, c, h, w = x.shape
    hw = h * w
    xf = np.ascontiguousarray(x.reshape(b, c, hw)).astype(ml_dtypes.bfloat16)
    wq, wk, wv, woA, woB = _prep_weights(w_qkv, w_out, b_out)

    in_maps = []
    for core in range(8):
        bi, qi = core // 4, core % 4
        # permute key pieces so piece 0 is this core's query chunk
        order = [qi] + [j for j in range(4) if j != qi]
        xb = np.ascontiguousarray(
            np.concatenate([xf[bi][:, 1024 * j:1024 * (j + 1)] for j in order],
                           axis=1))
        in_maps.append({
            "x": xb,
            "wq": wq, "wk": wk, "wv": wv, "woA": woA, "woB": woB,
        })

    nc = _get_nc()
    res = run_bass_kernel_spmd(nc, in_maps, core_ids=list(range(8)))
    y = np.empty((b, c, hw), np.float32)
    for core in range(8):
        bi, qi = core // 4, core % 4
        y[bi, :, QC * qi:QC * (qi + 1)] = res.results[core]["out"]
    return y.reshape(b, c, h, w)
